# revision 19
# baseline (speedup 1.0000x reference)
"""HMM window log-likelihood on 8 NeuronCores (data-parallel over batch).

Math: reference computes, per batch column b,
    y[b] = exp(logsumexp_i x_T[b,i]),  x via log-space forward recursion.
Equivalently in linear space with row-normalized transitions W_t and
emission table L = softmax(distros, axis=1), evaluated MEET-IN-THE-MIDDLE
so the serial dependence is ~128 steps instead of 255:
    forward   x_0 = em_0;  x_t = em_t . (Wf_t^T x_{t-1}),   t = 1..127
    backward  beta_255 = em_255;  beta_{t-1} = Wb_t^T (em_t . beta_t),
                                                            t = 255..128
    y[b] = sum_k x_127[k,b] * beta_127[k,b]
Wf/Wb carry the row-normalization and per-step rescale factors gf/gb
(host-computed from batch column 0 in f64) folded into their entries.
The emission table em[k,t,b] = L[k, bin(b,t)] is a host-side gather
streamed to SBUF as bf16 over DMA (keeps the PE free for matmuls).
Per step each direction costs one 512-wide PE matmul and one DVE
tensor-multiply (the multiply reads its beta/x operand straight from
PSUM; the two directions dovetail on the two engines).
Device returns colsum[b] = y[b] * prod(g); host: lnY = log(colsum)+C.
The true lnY is ~ -584.6 for these inputs, so y underflows f32 to 0.0 —
exactly matching the reference (which also underflows in f32).
"""
import sys, os
for p in ("/opt/trn_rl_repo",):
    if p not in sys.path:
        sys.path.insert(0, p)
import numpy as np
import ml_dtypes

from concourse import bass, bacc, mybir
from concourse.tile import TileContext
from concourse.bass_utils import run_bass_kernel_spmd

W, L, B, NB = 128, 256, 4096, 10
NCORES = 8
BC = B // NCORES          # 512 batch cols per core
M = 127                   # forward covers t=0..M, backward t=255..M+1
TBLK = 8                  # em streaming block (t's per DMA)

LAST_LNY = None           # debug: device-derived lnY per batch col
LAST_RESULTS = None       # debug: raw BassKernelResults

_CACHED = None            # (nc,) build cache


def _build_nc():
    nc = bacc.Bacc("TRN2", target_bir_lowering=False, debug=False,
                   num_devices=NCORES)
    bf16, f32 = mybir.dt.bfloat16, mybir.dt.float32

    wtb = nc.dram_tensor("wtb", [W, L - M - 1, W], bf16, kind="ExternalInput")
    wtf = nc.dram_tensor("wtf", [W, M, W], bf16, kind="ExternalInput")
    em = nc.dram_tensor("em", [W, L, BC], bf16, kind="ExternalInput")
    ones = nc.dram_tensor("ones", [W, 1], bf16, kind="ExternalInput")
    colsum = nc.dram_tensor("colsum", [1, BC], f32, kind="ExternalOutput")

    with TileContext(nc) as tc:
        with tc.sbuf_pool(name="sb", bufs=2) as sb, \
                tc.psum_pool(name="ps", bufs=2) as ps:
            ones_sb = sb.tile([W, 1], bf16, bufs=1)
            nc.sync.dma_start(ones_sb, ones.ap())

            em_tiles = {}

            def ensure_em(blk):
                if 0 <= blk < L // TBLK and blk not in em_tiles:
                    tag = "emf" if blk < (M + 1) // TBLK else "emb"
                    et = sb.tile([W, TBLK, BC], bf16, tag=tag, bufs=4)
                    nc.sync.dma_start(
                        et, em.ap()[:, blk * TBLK:(blk + 1) * TBLK, :])
                    em_tiles[blk] = et

            def em_slice(t, ahead):
                blk, ti = t // TBLK, t % TBLK
                ensure_em(blk)
                ensure_em(blk + ahead)  # prefetch next block in scan order
                return em_tiles[blk][:, ti, :]

            # issue all streaming DMAs upfront, interleaved in consumption
            # order: both em streams and both weight streams advance 8 t's
            # per unit, so strict alternation keeps every stream just ahead
            # of compute (late units WAR-block on pool buffers, which is
            # fine: everything behind them in the queue is needed later)
            wtb_sb = sb.tile([W, L - M - 1, W], bf16, bufs=1)
            wtf_sb = sb.tile([W, M, W], bf16, bufs=1)
            nb_ch = (L - M - 1 + 7) // 8
            nf_ch = (M + 7) // 8
            for k in range(L // TBLK // 2):
                ensure_em(L // TBLK - 1 - k)
                ensure_em(k)
                if k < nb_ch:
                    t0 = (nb_ch - 1 - k) * 8
                    cnt = min(8, L - M - 1 - t0)
                    nc.sync.dma_start(wtb_sb[:, t0:t0 + cnt, :],
                                      wtb.ap()[:, t0:t0 + cnt, :])
                if k < nf_ch:
                    t0 = k * 8
                    cnt = min(8, M - t0)
                    nc.sync.dma_start(wtf_sb[:, t0:t0 + cnt, :],
                                      wtf.ap()[:, t0:t0 + cnt, :])

            # s = 0 boundary: backward starts from c = em_255, forward from
            # x_prev = em_0
            c_b = em_slice(L - 1, -1)
            b_ps = ps.tile([W, BC], f32, tag="bb", bufs=2)
            nc.tensor.matmul(b_ps, wtb_sb[:, L - M - 2, :], c_b,
                             start=True, stop=True)
            x_sb = em_slice(0, 1)

            for s in range(1, M + 1):
                tb = L - 1 - s          # 254..128
                tf = s                  # 1..127
                # forward matmul first so the PE works while the backward
                # multiply is still draining
                xh_ps = ps.tile([W, BC], f32, tag="xh", bufs=2)
                nc.tensor.matmul(xh_ps, wtf_sb[:, tf - 1, :], x_sb,
                                 start=True, stop=True)
                c = sb.tile([W, BC], bf16, tag="cb", bufs=3)
                nc.vector.tensor_mul(c, em_slice(tb, -1), b_ps)
                nb = ps.tile([W, BC], f32, tag="bb", bufs=2)
                nc.tensor.matmul(nb, wtb_sb[:, tb - M - 1, :], c,
                                 start=True, stop=True)
                b_ps = nb
                x = sb.tile([W, BC], bf16, tag="xf", bufs=3)
                nc.vector.tensor_mul(x, em_slice(tf, 1), xh_ps)
                x_sb = x

            # merge: y = sum_k x_127 . beta_127
            prod = sb.tile([W, BC], bf16, tag="pr", bufs=1)
            nc.vector.tensor_mul(prod, x_sb, b_ps)
            cs_ps = ps.tile([1, BC], f32, tag="cs", bufs=1)
            nc.tensor.matmul(cs_ps, ones_sb, prod, start=True, stop=True)
            cs_sb = sb.tile([1, BC], f32, bufs=1)
            nc.vector.tensor_copy(cs_sb, cs_ps)
            nc.sync.dma_start(colsum.ap(), cs_sb)
    nc.compile()
    return nc


def _host_prep(data, input_distros, dense_layer_weights):
    f64 = np.float64
    we = np.exp(dense_layer_weights.astype(f64))           # (255,W,W)
    recip = 1.0 / we.sum(axis=2)                           # (255,W)
    d = input_distros.astype(f64)
    d = d - d.max(axis=1, keepdims=True)
    e = np.exp(d)
    Ll = e / e.sum(axis=1, keepdims=True)                  # (W,NB) softmax rows
    # bins exactly as reference: floor(v / 0.1) in f32
    bins = np.minimum(NB - 1, np.floor(
        data / np.float32(0.1)).astype(np.int32))          # (B,L)

    # batch-column-0 f64 passes -> per-step rescales gf/gb, offset C
    x = Ll[:, bins[0, 0]].copy()
    Cf = 0.0
    gf = np.ones(L, f64)
    for t in range(1, M + 1):
        xh = (we[t - 1] * recip[t - 1][:, None]) @ x
        xh = xh * Ll[:, bins[0, t]]
        f = xh.max()
        gf[t] = 1.0 / f
        Cf += np.log(f)
        x = xh * gf[t]
    beta = np.ones(W, dtype=f64)
    Cb = 0.0
    gb = np.ones(L, f64)
    for t in range(L - 1, M, -1):
        c = Ll[:, bins[0, t]] * beta * recip[t - 1]
        tmp = we[t - 1].T @ c
        f = tmp.max()
        gb[t] = 1.0 / f
        Cb += np.log(f)
        beta = tmp * gb[t]

    # fold normalization + rescales into the transition weights.
    # backward mm at t (128..255): contracts partition k (rows of we[t-1]):
    #   wtb[k, t-128, i] = we[t-1, k, i] * recip[t-1, k] * gb[t]
    Ab = we[M:] * (recip[M:, :, None] * gb[M + 1:, None, None])
    wtb = np.ascontiguousarray(Ab.transpose(1, 0, 2)).astype(ml_dtypes.bfloat16)
    # forward mm at t (1..127): x_t[i] = em*sum_j W[i,j]x[j]:
    #   wtf[j, t-1, i] = we[t-1, i, j] * recip[t-1, i] * gf[t]
    Af = we[:M] * (recip[:M, :, None] * gf[1:M + 1, None, None])
    wtf = np.ascontiguousarray(Af.transpose(2, 0, 1)).astype(ml_dtypes.bfloat16)

    # emission table gather, per core slice: em[k, t, b] = L[k, bin(b,t)]
    Lb = Ll.astype(ml_dtypes.bfloat16)                     # (W, NB)
    ems = []
    for c in range(NCORES):
        bc = bins[c * BC:(c + 1) * BC, :].T                # (L, BC)
        ems.append(Lb[:, bc])                              # (W, L, BC)
    ones_v = np.ones((W, 1), dtype=ml_dtypes.bfloat16)
    return wtb, wtf, ems, ones_v, Cf + Cb


def kernel(data, input_distros, dense_layer_weights):
    global LAST_LNY, LAST_RESULTS, _CACHED
    wtb, wtf, ems, ones_v, Cacc = _host_prep(
        np.asarray(data), np.asarray(input_distros),
        np.asarray(dense_layer_weights))

    if _CACHED is None:
        _CACHED = _build_nc()
    nc = _CACHED

    in_maps = [{"wtb": wtb, "wtf": wtf, "em": ems[c], "ones": ones_v}
               for c in range(NCORES)]
    res = run_bass_kernel_spmd(
        nc, in_maps, core_ids=list(range(NCORES)),
        trace=bool(int(os.environ.get("KERNEL_TRACE", "0"))),
        tmpdir=os.environ.get("KERNEL_TRACE_DIR") or None)
    LAST_RESULTS = res
    cs = np.concatenate([res.results[c]["colsum"].reshape(-1)
                         for c in range(NCORES)])           # (B,)
    lnY = np.log(cs.astype(np.float64)) + Cacc
    LAST_LNY = lnY
    y = np.exp(lnY).astype(np.float32).reshape(B, 1)
    return y


# revision 20
# speedup vs baseline: 1.1484x; 1.1484x over previous
"""HMM window log-likelihood on 8 NeuronCores (data-parallel over batch).

Math: reference computes, per batch column b,
    y[b] = exp(logsumexp_i x_T[b,i]),  x via log-space forward recursion.
Equivalently in linear space with row-normalized transitions W_t and
emission table L = softmax(distros, axis=1), evaluated MEET-IN-THE-MIDDLE
so the serial dependence is ~128 steps instead of 255:
    forward   x_0 = em_0;  x_t = em_t . (Wf_t^T x_{t-1}),   t = 1..127
    backward  beta_255 = em_255;  beta_{t-1} = Wb_t^T (em_t . beta_t),
                                                            t = 255..128
    y[b] = sum_k x_127[k,b] * beta_127[k,b]
Wf/Wb carry the row-normalization and per-step rescale factors gf/gb
(host-computed from batch column 0 in f64) folded into their entries.
The emission table em[k,t,b] = L[k, bin(b,t)] is a host-side gather
streamed to SBUF as bf16 over DMA (keeps the PE free for matmuls).
Per step each direction costs one 512-wide PE matmul and one DVE
tensor-multiply (the multiply reads its beta/x operand straight from
PSUM; the two directions dovetail on the two engines).
Device returns colsum[b] = y[b] * prod(g); host: lnY = log(colsum)+C.
The true lnY is ~ -584.6 for these inputs, so y underflows f32 to 0.0 —
exactly matching the reference (which also underflows in f32).
"""
import sys, os
for p in ("/opt/trn_rl_repo",):
    if p not in sys.path:
        sys.path.insert(0, p)
import numpy as np
import ml_dtypes

from concourse import bass, bacc, mybir
from concourse.tile import TileContext
from concourse.bass_utils import run_bass_kernel_spmd

W, L, B, NB = 128, 256, 4096, 10
NCORES = 8
BC = B // NCORES          # 512 batch cols per core
M = 127                   # forward covers t=0..M, backward t=255..M+1
TBLK = 8                  # em streaming block (t's per DMA)

LAST_LNY = None           # debug: device-derived lnY per batch col
LAST_RESULTS = None       # debug: raw BassKernelResults

_CACHED = None            # (nc,) build cache


def _build_nc():
    nc = bacc.Bacc("TRN2", target_bir_lowering=False, debug=False,
                   num_devices=NCORES)
    bf16, f32 = mybir.dt.bfloat16, mybir.dt.float32

    wtb = nc.dram_tensor("wtb", [W, L - M - 1, W], bf16, kind="ExternalInput")
    wtf = nc.dram_tensor("wtf", [W, M, W], bf16, kind="ExternalInput")
    em = nc.dram_tensor("em", [W, L, BC], bf16, kind="ExternalInput")
    ones = nc.dram_tensor("ones", [W, 1], bf16, kind="ExternalInput")
    colsum = nc.dram_tensor("colsum", [1, BC], f32, kind="ExternalOutput")

    with TileContext(nc) as tc:
        with tc.sbuf_pool(name="sb", bufs=2) as sb, \
                tc.psum_pool(name="ps", bufs=2) as ps:
            ones_sb = sb.tile([W, 1], bf16, bufs=1)
            nc.sync.dma_start(ones_sb, ones.ap())

            em_tiles = {}

            def ensure_em(blk):
                if 0 <= blk < L // TBLK and blk not in em_tiles:
                    tag = "emf" if blk < (M + 1) // TBLK else "emb"
                    et = sb.tile([W, TBLK, BC], bf16, tag=tag, bufs=4)
                    nc.sync.dma_start(
                        et, em.ap()[:, blk * TBLK:(blk + 1) * TBLK, :])
                    em_tiles[blk] = et

            def em_slice(t, ahead):
                blk, ti = t // TBLK, t % TBLK
                ensure_em(blk)
                ensure_em(blk + ahead)      # prefetch 2 blocks in scan order
                ensure_em(blk + 2 * ahead)
                return em_tiles[blk][:, ti, :]

            # DMA issue order: the head em blocks and head weight chunks of
            # each direction gate the first compute, so they go first; the
            # remaining weight bulk follows (its deadlines are late); later
            # em blocks are issued from inside the loop, 2 blocks ahead.
            wtb_sb = sb.tile([W, L - M - 1, W], bf16, bufs=1)
            wtf_sb = sb.tile([W, M, W], bf16, bufs=1)
            nb_ch = (L - M - 1 + 7) // 8
            nf_ch = (M + 7) // 8

            def wtb_chunk(k):
                t0 = (nb_ch - 1 - k) * 8
                cnt = min(8, L - M - 1 - t0)
                nc.sync.dma_start(wtb_sb[:, t0:t0 + cnt, :],
                                  wtb.ap()[:, t0:t0 + cnt, :])

            def wtf_chunk(k):
                t0 = k * 8
                cnt = min(8, M - t0)
                nc.sync.dma_start(wtf_sb[:, t0:t0 + cnt, :],
                                  wtf.ap()[:, t0:t0 + cnt, :])

            nblk = L // TBLK
            ensure_em(nblk - 1), ensure_em(0)
            wtb_chunk(0), wtf_chunk(0)
            ensure_em(nblk - 2), ensure_em(1)
            wtb_chunk(1), wtf_chunk(1)
            for k in (2, 3):
                ensure_em(nblk - 1 - k), ensure_em(k)
            for k in range(2, max(nb_ch, nf_ch)):
                if k < nb_ch:
                    wtb_chunk(k)
                if k < nf_ch:
                    wtf_chunk(k)

            # s = 0 boundary: backward starts from c = em_255, forward from
            # x_prev = em_0
            c_b = em_slice(L - 1, -1)
            b_ps = ps.tile([W, BC], f32, tag="bb", bufs=2)
            nc.tensor.matmul(b_ps, wtb_sb[:, L - M - 2, :], c_b,
                             start=True, stop=True)
            x_sb = em_slice(0, 1)

            for s in range(1, M + 1):
                tb = L - 1 - s          # 254..128
                tf = s                  # 1..127
                # forward matmul first so the PE works while the backward
                # multiply is still draining
                xh_ps = ps.tile([W, BC], f32, tag="xh", bufs=2)
                nc.tensor.matmul(xh_ps, wtf_sb[:, tf - 1, :], x_sb,
                                 start=True, stop=True)
                c = sb.tile([W, BC], bf16, tag="cb", bufs=3)
                nc.vector.tensor_mul(c, em_slice(tb, -1), b_ps)
                nb = ps.tile([W, BC], f32, tag="bb", bufs=2)
                nc.tensor.matmul(nb, wtb_sb[:, tb - M - 1, :], c,
                                 start=True, stop=True)
                b_ps = nb
                x = sb.tile([W, BC], bf16, tag="xf", bufs=3)
                nc.vector.tensor_mul(x, em_slice(tf, 1), xh_ps)
                x_sb = x

            # merge: y = sum_k x_127 . beta_127
            prod = sb.tile([W, BC], bf16, tag="pr", bufs=1)
            nc.vector.tensor_mul(prod, x_sb, b_ps)
            cs_ps = ps.tile([1, BC], f32, tag="cs", bufs=1)
            nc.tensor.matmul(cs_ps, ones_sb, prod, start=True, stop=True)
            cs_sb = sb.tile([1, BC], f32, bufs=1)
            nc.vector.tensor_copy(cs_sb, cs_ps)
            nc.sync.dma_start(colsum.ap(), cs_sb)
    nc.compile()
    return nc


def _host_prep(data, input_distros, dense_layer_weights):
    f64 = np.float64
    we = np.exp(dense_layer_weights.astype(f64))           # (255,W,W)
    recip = 1.0 / we.sum(axis=2)                           # (255,W)
    d = input_distros.astype(f64)
    d = d - d.max(axis=1, keepdims=True)
    e = np.exp(d)
    Ll = e / e.sum(axis=1, keepdims=True)                  # (W,NB) softmax rows
    # bins exactly as reference: floor(v / 0.1) in f32
    bins = np.minimum(NB - 1, np.floor(
        data / np.float32(0.1)).astype(np.int32))          # (B,L)

    # batch-column-0 f64 passes -> per-step rescales gf/gb, offset C
    x = Ll[:, bins[0, 0]].copy()
    Cf = 0.0
    gf = np.ones(L, f64)
    for t in range(1, M + 1):
        xh = (we[t - 1] * recip[t - 1][:, None]) @ x
        xh = xh * Ll[:, bins[0, t]]
        f = xh.max()
        gf[t] = 1.0 / f
        Cf += np.log(f)
        x = xh * gf[t]
    beta = np.ones(W, dtype=f64)
    Cb = 0.0
    gb = np.ones(L, f64)
    for t in range(L - 1, M, -1):
        c = Ll[:, bins[0, t]] * beta * recip[t - 1]
        tmp = we[t - 1].T @ c
        f = tmp.max()
        gb[t] = 1.0 / f
        Cb += np.log(f)
        beta = tmp * gb[t]

    # fold normalization + rescales into the transition weights.
    # backward mm at t (128..255): contracts partition k (rows of we[t-1]):
    #   wtb[k, t-128, i] = we[t-1, k, i] * recip[t-1, k] * gb[t]
    Ab = we[M:] * (recip[M:, :, None] * gb[M + 1:, None, None])
    wtb = np.ascontiguousarray(Ab.transpose(1, 0, 2)).astype(ml_dtypes.bfloat16)
    # forward mm at t (1..127): x_t[i] = em*sum_j W[i,j]x[j]:
    #   wtf[j, t-1, i] = we[t-1, i, j] * recip[t-1, i] * gf[t]
    Af = we[:M] * (recip[:M, :, None] * gf[1:M + 1, None, None])
    wtf = np.ascontiguousarray(Af.transpose(2, 0, 1)).astype(ml_dtypes.bfloat16)

    # emission table gather, per core slice: em[k, t, b] = L[k, bin(b,t)]
    Lb = Ll.astype(ml_dtypes.bfloat16)                     # (W, NB)
    ems = []
    for c in range(NCORES):
        bc = bins[c * BC:(c + 1) * BC, :].T                # (L, BC)
        ems.append(Lb[:, bc])                              # (W, L, BC)
    ones_v = np.ones((W, 1), dtype=ml_dtypes.bfloat16)
    return wtb, wtf, ems, ones_v, Cf + Cb


def kernel(data, input_distros, dense_layer_weights):
    global LAST_LNY, LAST_RESULTS, _CACHED
    wtb, wtf, ems, ones_v, Cacc = _host_prep(
        np.asarray(data), np.asarray(input_distros),
        np.asarray(dense_layer_weights))

    if _CACHED is None:
        _CACHED = _build_nc()
    nc = _CACHED

    in_maps = [{"wtb": wtb, "wtf": wtf, "em": ems[c], "ones": ones_v}
               for c in range(NCORES)]
    res = run_bass_kernel_spmd(
        nc, in_maps, core_ids=list(range(NCORES)),
        trace=bool(int(os.environ.get("KERNEL_TRACE", "0"))),
        tmpdir=os.environ.get("KERNEL_TRACE_DIR") or None)
    LAST_RESULTS = res
    cs = np.concatenate([res.results[c]["colsum"].reshape(-1)
                         for c in range(NCORES)])           # (B,)
    lnY = np.log(cs.astype(np.float64)) + Cacc
    LAST_LNY = lnY
    y = np.exp(lnY).astype(np.float32).reshape(B, 1)
    return y
